# revision 7
# baseline (speedup 1.0000x reference)
"""Trainium2 Bass kernel for nn_CANN_39994735460546.

Reference semantics:
  t    = (physical_params[:, :, None] ** PS_POWERS).reshape(B, 64)
  norm = (t - t.mean()) / t.std(ddof=1)          # global scalar stats
  h    = relu(norm) @ W1.T + b1
  c    = h @ W2.T + b2                            # [B, 5]
  dy[b, j] = sum_k c[b,k] * p_k * eta[b,j]^(p_k - 1),  p = [2,5,8,11,14]
           = eta * P(u),  u = eta^3,  P = quartic with per-row coefficients.

Device strategy (8 NeuronCores, pure data parallel over eta rows; each core
owns 512 rows = 4 tiles of 128):

  Stage 1 ([B,4] -> per-row poly coefficients) is a ~130 KFLOP
  input-only transform; it runs on host in f64 where we also factor the
  quartic into two real quadratics (always possible over R):

    S*P(u) = (g1*(a1*u+b1)^2 + d1) * (g2*(a2*u+b2)^2 + d2)

  with per-row scales chosen so every f16 intermediate stays in range
  (product capped at 3e4, ACT square outputs at ~1e3); S is undone on
  host. Validated vs reference on the real inputs: rel err 2.7e-3 (f64
  factorization error 2e-7), vs 4.7e-3 for the previous Horner kernel.

  Stage 2 per 128x4096 tile, balanced across ACT and DVE (ACT runs 1x at
  1.2 GHz; DVE tensor_tensor 2x / tensor_scalar 4x at 0.96 GHz):
    ACT: s = eta^2 on cols [0,ACOLS) (Square); sq_i = Square(a_i*u + b_i)
         (the free affine absorbs the quadratic's shift), i = 1,2.
    DVE: s on cols [ACOLS,CT) (tt); u = s*eta (tt); F_i = sq_i*g_i + d_i
         (ts mult-add, 4x); G = F1*F2 (tt); dy = G*eta (tt -> bf16).
  That's 3 tt + 2 ts on DVE (8192 cyc) vs Horner's 5 tt + 4 ts (14336).
  ~9.7 us/engine/tile, both engines ~equally busy.

  eta ships as f16 (host cast): halves load traffic (1 MB/tile) and
  makes s the only derived power; dy stores as bf16. The per-tile
  emission is software-pipelined with skew 2 (ACT squares of tile g-1,
  DVE F-chain of tile g-2) so the u -> sq -> F cross-engine cycle spans
  two iterations and never binds; loads prefetch via pool depth.
"""

import sys
import numpy as np

sys.path.insert(0, "/opt/trn_rl_repo")

B = 4096
L = 4096
NCORES = 8
RPC = B // NCORES          # rows per core = 512
NPT = RPC // 128           # 128-row tiles per core = 4
CT = 4096                  # row width
ACOLS = 2688               # columns of s = eta^2 computed on ACT (rest DVE)
UNROLL = 16                # stage-2 passes per hardware-loop iteration

PS_POWERS = np.array([-5.0, -4.0, -3.0, -2.0, -1.5, -1.0, -0.5, 0.0,
                      0.5, 2.0, 1.0 / 3.0, 3.0, 0.25, 4.0, 0.2, 5.0],
                     dtype=np.float64)
POLY_POWERS = np.array([2.0, 5.0, 8.0, 11.0, 14.0], dtype=np.float64)

_cache = {}


def _build_nc(repeat=1, force_unroll=False):
    import concourse.bass as bass
    import concourse.tile as tile
    from concourse import bacc, mybir

    F32 = mybir.dt.float32
    BF16 = mybir.dt.bfloat16
    F16 = mybir.dt.float16
    AF = mybir.ActivationFunctionType
    OP = mybir.AluOpType

    nc = bacc.Bacc("TRN2", target_bir_lowering=False, debug=False,
                   num_devices=NCORES)

    eta_d = nc.dram_tensor("eta", [RPC, L], F16, kind="ExternalInput").ap()
    cf_d = nc.dram_tensor("cf", [RPC, 8], F32, kind="ExternalInput").ap()
    dy_d = nc.dram_tensor("dy", [RPC, L], BF16, kind="ExternalOutput").ap()

    from contextlib import ExitStack

    with tile.TileContext(nc) as tc, ExitStack() as stack:
        p_const = stack.enter_context(tc.tile_pool(name="consts", bufs=1))
        p_eta = stack.enter_context(tc.tile_pool(name="eta", bufs=5))
        p_s = stack.enter_context(tc.tile_pool(name="s", bufs=2))
        p_u = stack.enter_context(tc.tile_pool(name="u", bufs=3))
        p_q1 = stack.enter_context(tc.tile_pool(name="q1", bufs=3))
        p_q2 = stack.enter_context(tc.tile_pool(name="q2", bufs=3))
        p_dy = stack.enter_context(tc.tile_pool(name="dy", bufs=2))

        # per-tile coefficient columns: a1 b1 g1 d1 a2 b2 g2 d2
        cfs = []
        for t in range(NPT):
            cf_t = p_const.tile([128, 8], F32, tag=f"cf{t}", name=f"cf{t}")
            nc.sync.dma_start(cf_t[:], cf_d[t * 128:(t + 1) * 128, :])
            cfs.append(cf_t)

        # rolling tile state for the software pipeline, keyed by global
        # tile index g (g = pass*NPT + t); skew: ACT squares lag 1, DVE
        # F-chain and the store lag 2.
        state = {}

        def emit(g, n):
            # --- load + s of tile g (head of the SP/ACT/DVE queues: s_a
            # must finish early in ACT's iteration, else DVE stalls at
            # u(g); the eta load prefetches via pool depth) ---
            if g < n:
                t = g % NPT
                rows = slice(t * 128, (t + 1) * 128)
                eta_t = p_eta.tile([128, CT], F16, tag="eta", name="eta_t")
                nc.sync.dma_start(eta_t[:], eta_d[rows, :])
                s_t = p_s.tile([128, CT], F16, tag="s", name="s_t")
                nc.scalar.activation(s_t[:, 0:ACOLS], eta_t[:, 0:ACOLS],
                                     AF.Square)
                if ACOLS < CT:
                    nc.vector.tensor_tensor(s_t[:, ACOLS:], eta_t[:, ACOLS:],
                                            eta_t[:, ACOLS:], OP.mult)
                state[g] = {"eta": eta_t, "s": s_t}

            # --- DVE: F-chain of tile g-2 ---
            if 0 <= g - 2 < n:
                st = state[g - 2]
                t = (g - 2) % NPT
                cf = cfs[t]
                sq1, sq2, eta_t = st["sq1"], st["sq2"], st["eta"]
                # F_i = sq_i * g_i + d_i, in place over sq_i
                nc.vector.tensor_scalar(sq1[:], sq1[:], cf[:, 2:3],
                                        cf[:, 3:4], OP.mult, OP.add)
                nc.vector.tensor_scalar(sq2[:], sq2[:], cf[:, 6:7],
                                        cf[:, 7:8], OP.mult, OP.add)
                nc.vector.tensor_tensor(sq1[:], sq1[:], sq2[:], OP.mult)
                dy_t = p_dy.tile([128, CT], BF16, tag="dy", name="dy_t")
                nc.vector.tensor_tensor(dy_t[:], sq1[:], eta_t[:], OP.mult)
                rows = slice(t * 128, (t + 1) * 128)
                st["store"] = (dy_d[rows, :], dy_t[:])

            # --- ACT: squares of tile g-1 ---
            if 0 <= g - 1 < n:
                st = state[g - 1]
                cf = cfs[(g - 1) % NPT]
                u_t = st["u"]
                sq1 = p_q1.tile([128, CT], F16, tag="sq1", name="sq1_t")
                nc.scalar.activation(sq1[:], u_t[:], AF.Square,
                                     scale=cf[:, 0:1], bias=cf[:, 1:2])
                sq2 = p_q2.tile([128, CT], F16, tag="sq2", name="sq2_t")
                nc.scalar.activation(sq2[:], u_t[:], AF.Square,
                                     scale=cf[:, 4:5], bias=cf[:, 5:6])
                st["sq1"], st["sq2"] = sq1, sq2

            # --- DVE tail: u of tile g (by now s_a(g) is long done) ---
            if g < n:
                st = state[g]
                u_t = p_u.tile([128, CT], F16, tag="u", name="u_t")
                nc.vector.tensor_tensor(u_t[:], st.pop("s")[:], st["eta"][:],
                                        OP.mult)
                st["u"] = u_t

            # --- store of tile g-2 (tail of the ACT queue: dy is long
            # since computed, so the HWDGE wait never stalls ACT) ---
            if 0 <= g - 2 < n:
                st = state.pop(g - 2)
                nc.scalar.dma_start(*st.pop("store"))

        def run_block(npass):
            n = npass * NPT
            for g in range(n + 2):
                emit(g, n)
            state.clear()

        if repeat <= UNROLL or force_unroll:
            run_block(repeat)
        else:
            # hardware loop: constant NEFF size for any repeat count;
            # UNROLL passes per iteration amortize the per-iteration
            # all-engine barrier and pipeline refill.
            n_iter, rem = divmod(repeat, UNROLL)
            with tc.For_i(0, n_iter):
                run_block(UNROLL)
            if rem:
                run_block(rem)
    nc.compile()
    return nc


def _stage1_coeffs(physical_params, W1, b1, W2, b2):
    """Exact stage 1 in f64: per-row coefficients of P(u) = sum_k cp_k u^k."""
    pp = np.asarray(physical_params, np.float64)
    t = (pp[:, :, None] ** PS_POWERS.reshape(1, 1, -1)).reshape(pp.shape[0], -1)
    norm = (t - t.mean()) / t.std(ddof=1)
    h = np.maximum(norm, 0.0) @ np.asarray(W1, np.float64).T \
        + np.asarray(b1, np.float64)
    c = h @ np.asarray(W2, np.float64).T + np.asarray(b2, np.float64)
    return c * POLY_POWERS.reshape(1, -1)


def _factor_quartics(cp):
    """P/c4 = ((u+h1)^2+r1)((u+h2)^2+r2) per row (real quadratics)."""
    n = cp.shape[0]
    mon = cp / cp[:, 4:5]
    comp = np.zeros((n, 4, 4))
    comp[:, 1, 0] = comp[:, 2, 1] = comp[:, 3, 2] = 1.0
    comp[:, 0, :] = -mon[:, [3, 2, 1, 0]]
    roots = np.linalg.eigvals(comp)
    h = np.empty((n, 2))
    r = np.empty((n, 2))
    for i in range(n):
        rt = roots[i]
        im = np.abs(rt.imag) > 1e-9 * (np.abs(rt.real) + 1.0)
        quads = []
        cplx = rt[im]
        used = np.zeros(len(cplx), bool)
        for j in range(len(cplx)):
            if used[j]:
                continue
            k = int(np.argmin(np.abs(cplx - np.conj(cplx[j])) + used * 1e18))
            used[j] = used[k] = True
            quads.append((-cplx[j].real, cplx[j].imag ** 2))
        real = rt[~im].real
        real = real[np.argsort(np.abs(real))]
        for j in range(0, len(real), 2):
            a, b = real[j], real[j + 1]
            m = (a + b) / 2.0
            quads.append((-m, a * b - m * m))
        h[i] = [quads[0][0], quads[1][0]]
        r[i] = [quads[0][1], quads[1][1]]
    return h[:, 0], r[:, 0], h[:, 1], r[:, 1]


def _pick_scales(c4, h1, r1, h2, r2, ulo, uhi, gmax=30000.0, sqmax=1024.0):
    """Per-row (a1,b1,g1,d1,a2,b2,g2,d2), f16-safe, and the row scale S."""
    def qabsmax(hh, rr):
        e0 = (ulo + hh) ** 2 + rr
        e1 = (uhi + hh) ** 2 + rr
        vtx = np.where((-hh >= ulo) & (-hh <= uhi), rr, e0)
        return np.maximum(np.abs(vtx), np.maximum(np.abs(e0), np.abs(e1)))

    M1 = qabsmax(h1, r1)
    M2 = qabsmax(h2, r2)
    S = np.minimum(1.0, gmax / (np.abs(c4) * M1 * M2))
    g1 = np.sign(c4) * np.sqrt(np.abs(c4) * S * M2 / M1)
    g2 = np.sqrt(np.abs(c4) * S * M1 / M2)

    def sqpeak(hh):
        return np.maximum((ulo + hh) ** 2, (uhi + hh) ** 2)

    a1 = np.sqrt(np.minimum(1.0, sqmax / sqpeak(h1)))
    a2 = np.sqrt(np.minimum(1.0, sqmax / sqpeak(h2)))
    cf = np.stack([a1, a1 * h1, g1 / a1 ** 2, g1 * r1,
                   a2, a2 * h2, g2 / a2 ** 2, g2 * r2], axis=1)
    return cf.astype(np.float32), S


def _host_prep(physical_params, eta, W1, b1, W2, b2):
    """Returns (eta_f16, cf [B,8] f32, S [B] f64)."""
    eta = np.asarray(eta, np.float32)
    cp = _stage1_coeffs(physical_params, W1, b1, W2, b2)
    h1, r1, h2, r2 = _factor_quartics(cp)
    ulo = float(eta.min()) ** 3
    uhi = float(eta.max()) ** 3
    cf, S = _pick_scales(cp[:, 4], h1, r1, h2, r2, ulo, uhi)
    return np.ascontiguousarray(eta.astype(np.float16)), cf, S


def _make_in_maps(eta16, cf):
    in_maps = []
    for g in range(NCORES):
        rows = slice(g * RPC, (g + 1) * RPC)
        in_maps.append({
            "eta": np.ascontiguousarray(eta16[rows]),
            "cf": np.ascontiguousarray(cf[rows]),
        })
    return in_maps


def kernel(physical_params, eta, W1, b1, W2, b2):
    from concourse.bass_utils import run_bass_kernel_spmd

    eta16, cf, S = _host_prep(physical_params, eta, W1, b1, W2, b2)

    if "nc" not in _cache:
        _cache["nc"] = _build_nc()
    nc = _cache["nc"]

    res = run_bass_kernel_spmd(nc, _make_in_maps(eta16, cf),
                               core_ids=list(range(NCORES)))
    _cache["last_results"] = res
    out = np.concatenate(
        [np.asarray(res.results[g]["dy"]).astype(np.float32)
         for g in range(NCORES)], axis=0)
    out /= S[:, None].astype(np.float32)
    return out


# revision 14
# speedup vs baseline: 1.3078x; 1.3078x over previous
"""Trainium2 Bass kernel for nn_CANN_39994735460546.

Reference semantics:
  t    = (physical_params[:, :, None] ** PS_POWERS).reshape(B, 64)
  norm = (t - t.mean()) / t.std(ddof=1)          # global scalar stats
  h    = relu(norm) @ W1.T + b1
  c    = h @ W2.T + b2                            # [B, 5]
  dy[b, j] = sum_k c[b,k] * p_k * eta[b,j]^(p_k - 1),  p = [2,5,8,11,14]
           = eta * P(u),  u = eta^3,  P = quartic with per-row coefficients.

Device strategy (8 NeuronCores, pure data parallel over eta rows; each core
owns 512 rows = 4 tiles of 128):

  Stage 1 ([B,4] -> per-row poly coefficients) is a ~130 KFLOP
  input-only transform; it runs on host in f64 where we also factor the
  quartic into two real quadratics (always possible over R):

    S*P(u) = (g1*(a1*u+b1)^2 + d1) * (g2*(a2*u+b2)^2 + d2)

  with per-row scales chosen so every f16 intermediate stays in range
  (product capped at 3e4, ACT square outputs at ~1e3); S is undone on
  host. Validated vs reference on the real inputs: rel err 2.7e-3 (f64
  factorization error 2e-7), vs 4.7e-3 for the previous Horner kernel.

  Stage 2 per 128x4096 tile, balanced across ACT and DVE (ACT runs 1x at
  1.2 GHz; DVE tensor_tensor 2x / tensor_scalar 4x at 0.96 GHz):
    ACT: s = eta^2 on cols [0,ACOLS) (Square); sq_i = Square(a_i*u + b_i)
         (the free affine absorbs the quadratic's shift), i = 1,2.
    DVE: s on cols [ACOLS,CT) (tt); u = s*eta (tt); F_i = sq_i*g_i + d_i
         (ts mult-add, 4x); G = F1*F2 (tt); dy = G*eta (tt -> bf16).
  That's 3 tt + 2 ts on DVE (8192 cyc) vs Horner's 5 tt + 4 ts (14336).
  ~9.7 us/engine/tile, both engines ~equally busy.

  eta ships as f16 (host cast): halves load traffic (1 MB/tile) and
  makes s the only derived power; dy stores as bf16. The per-tile
  emission is software-pipelined with skew 2 (ACT squares of tile g-1,
  DVE F-chain of tile g-2) so the u -> sq -> F cross-engine cycle spans
  two iterations and never binds; loads prefetch via pool depth.
"""

import sys
import numpy as np

sys.path.insert(0, "/opt/trn_rl_repo")

B = 4096
L = 4096
NCORES = 8
RPC = B // NCORES          # rows per core = 512
NPT = RPC // 128           # 128-row tiles per core = 4
CT = 4096                  # row width
ACOLS = 2816               # columns of s = eta^2 computed on ACT (rest DVE)
UNROLL = 64                # stage-2 passes per hardware-loop iteration
SA_FIRST = True            # emit load+s at the head of each iteration

PS_POWERS = np.array([-5.0, -4.0, -3.0, -2.0, -1.5, -1.0, -0.5, 0.0,
                      0.5, 2.0, 1.0 / 3.0, 3.0, 0.25, 4.0, 0.2, 5.0],
                     dtype=np.float64)
POLY_POWERS = np.array([2.0, 5.0, 8.0, 11.0, 14.0], dtype=np.float64)

_cache = {}


def _build_nc(repeat=1, force_unroll=False):
    import concourse.bass as bass
    import concourse.tile as tile
    from concourse import bacc, mybir

    F32 = mybir.dt.float32
    BF16 = mybir.dt.bfloat16
    F16 = mybir.dt.float16
    AF = mybir.ActivationFunctionType
    OP = mybir.AluOpType

    nc = bacc.Bacc("TRN2", target_bir_lowering=False, debug=False,
                   num_devices=NCORES)

    eta_d = nc.dram_tensor("eta", [RPC, L], F16, kind="ExternalInput").ap()
    cf_d = nc.dram_tensor("cf", [RPC, 8], F32, kind="ExternalInput").ap()
    dy_d = nc.dram_tensor("dy", [RPC, L], BF16, kind="ExternalOutput").ap()

    from contextlib import ExitStack

    with tile.TileContext(nc) as tc, ExitStack() as stack:
        p_const = stack.enter_context(tc.tile_pool(name="consts", bufs=1))
        p_eta = stack.enter_context(tc.tile_pool(name="eta", bufs=5))
        p_s = stack.enter_context(tc.tile_pool(name="s", bufs=2))
        p_u = stack.enter_context(tc.tile_pool(name="u", bufs=3))
        p_q1 = stack.enter_context(tc.tile_pool(name="q1", bufs=3))
        p_q2 = stack.enter_context(tc.tile_pool(name="q2", bufs=3))
        p_dy = stack.enter_context(tc.tile_pool(name="dy", bufs=3))

        # per-tile coefficient columns: a1 b1 g1 d1 a2 b2 g2 d2
        cfs = []
        for t in range(NPT):
            cf_t = p_const.tile([128, 8], F32, tag=f"cf{t}", name=f"cf{t}")
            nc.sync.dma_start(cf_t[:], cf_d[t * 128:(t + 1) * 128, :])
            cfs.append(cf_t)

        # rolling tile state for the software pipeline, keyed by global
        # tile index g (g = pass*NPT + t); skew: ACT squares lag 1, DVE
        # F-chain and the store lag 2.
        state = {}

        def load_s(g, n):
            # load + s of tile g (with SA_FIRST this heads the queues: s_a
            # finishes early in ACT's iteration so DVE's u(g) never waits;
            # the eta load prefetches via pool depth)
            if g < n:
                t = g % NPT
                rows = slice(t * 128, (t + 1) * 128)
                eta_t = p_eta.tile([128, CT], F16, tag="eta", name="eta_t")
                nc.sync.dma_start(eta_t[:], eta_d[rows, :])
                s_t = p_s.tile([128, CT], F16, tag="s", name="s_t")
                nc.scalar.activation(s_t[:, 0:ACOLS], eta_t[:, 0:ACOLS],
                                     AF.Square)
                if ACOLS < CT:
                    nc.vector.tensor_tensor(s_t[:, ACOLS:], eta_t[:, ACOLS:],
                                            eta_t[:, ACOLS:], OP.mult)
                state[g] = {"eta": eta_t, "s": s_t}

        def emit(g, n):
            # --- store of tile g-3 (head of ACT's queue: dy(g-3) completed
            # and drained an iteration ago, so the HWDGE wait is free) ---
            if 0 <= g - 3 < n:
                st = state.pop(g - 3)
                nc.scalar.dma_start(*st.pop("store"))

            if SA_FIRST:
                load_s(g, n)

            # --- DVE: F-chain of tile g-2 ---
            if 0 <= g - 2 < n:
                st = state[g - 2]
                t = (g - 2) % NPT
                cf = cfs[t]
                sq1, sq2, eta_t = st["sq1"], st["sq2"], st["eta"]
                # F_i = sq_i * g_i + d_i, in place over sq_i
                nc.vector.tensor_scalar(sq1[:], sq1[:], cf[:, 2:3],
                                        cf[:, 3:4], OP.mult, OP.add)
                nc.vector.tensor_scalar(sq2[:], sq2[:], cf[:, 6:7],
                                        cf[:, 7:8], OP.mult, OP.add)
                nc.vector.tensor_tensor(sq1[:], sq1[:], sq2[:], OP.mult)
                dy_t = p_dy.tile([128, CT], BF16, tag="dy", name="dy_t")
                nc.vector.tensor_tensor(dy_t[:], sq1[:], eta_t[:], OP.mult)
                rows = slice(t * 128, (t + 1) * 128)
                st["store"] = (dy_d[rows, :], dy_t[:])

            # --- ACT: squares of tile g-1 ---
            if 0 <= g - 1 < n:
                st = state[g - 1]
                cf = cfs[(g - 1) % NPT]
                u_t = st["u"]
                sq1 = p_q1.tile([128, CT], F16, tag="sq1", name="sq1_t")
                nc.scalar.activation(sq1[:], u_t[:], AF.Square,
                                     scale=cf[:, 0:1], bias=cf[:, 1:2])
                sq2 = p_q2.tile([128, CT], F16, tag="sq2", name="sq2_t")
                nc.scalar.activation(sq2[:], u_t[:], AF.Square,
                                     scale=cf[:, 4:5], bias=cf[:, 5:6])
                st["sq1"], st["sq2"] = sq1, sq2

            if not SA_FIRST:
                load_s(g, n)

            # --- DVE tail: u of tile g (by now s_a(g) is long done) ---
            if g < n:
                st = state[g]
                u_t = p_u.tile([128, CT], F16, tag="u", name="u_t")
                nc.vector.tensor_tensor(u_t[:], st.pop("s")[:], st["eta"][:],
                                        OP.mult)
                st["u"] = u_t

        def run_block(npass):
            n = npass * NPT
            for g in range(n + 3):
                emit(g, n)
            state.clear()

        if repeat <= UNROLL or force_unroll:
            run_block(repeat)
        else:
            # hardware loop: constant NEFF size for any repeat count;
            # UNROLL passes per iteration amortize the per-iteration
            # all-engine barrier and pipeline refill.
            n_iter, rem = divmod(repeat, UNROLL)
            with tc.For_i(0, n_iter):
                run_block(UNROLL)
            if rem:
                run_block(rem)
    nc.compile()
    return nc


def _stage1_coeffs(physical_params, W1, b1, W2, b2):
    """Exact stage 1 in f64: per-row coefficients of P(u) = sum_k cp_k u^k."""
    pp = np.asarray(physical_params, np.float64)
    t = (pp[:, :, None] ** PS_POWERS.reshape(1, 1, -1)).reshape(pp.shape[0], -1)
    norm = (t - t.mean()) / t.std(ddof=1)
    h = np.maximum(norm, 0.0) @ np.asarray(W1, np.float64).T \
        + np.asarray(b1, np.float64)
    c = h @ np.asarray(W2, np.float64).T + np.asarray(b2, np.float64)
    return c * POLY_POWERS.reshape(1, -1)


def _factor_quartics(cp):
    """P/c4 = ((u+h1)^2+r1)((u+h2)^2+r2) per row (real quadratics)."""
    n = cp.shape[0]
    mon = cp / cp[:, 4:5]
    comp = np.zeros((n, 4, 4))
    comp[:, 1, 0] = comp[:, 2, 1] = comp[:, 3, 2] = 1.0
    comp[:, 0, :] = -mon[:, [3, 2, 1, 0]]
    roots = np.linalg.eigvals(comp)
    h = np.empty((n, 2))
    r = np.empty((n, 2))
    for i in range(n):
        rt = roots[i]
        im = np.abs(rt.imag) > 1e-9 * (np.abs(rt.real) + 1.0)
        quads = []
        cplx = rt[im]
        used = np.zeros(len(cplx), bool)
        for j in range(len(cplx)):
            if used[j]:
                continue
            k = int(np.argmin(np.abs(cplx - np.conj(cplx[j])) + used * 1e18))
            used[j] = used[k] = True
            quads.append((-cplx[j].real, cplx[j].imag ** 2))
        real = rt[~im].real
        real = real[np.argsort(np.abs(real))]
        for j in range(0, len(real), 2):
            a, b = real[j], real[j + 1]
            m = (a + b) / 2.0
            quads.append((-m, a * b - m * m))
        h[i] = [quads[0][0], quads[1][0]]
        r[i] = [quads[0][1], quads[1][1]]
    return h[:, 0], r[:, 0], h[:, 1], r[:, 1]


def _pick_scales(c4, h1, r1, h2, r2, ulo, uhi, gmax=30000.0, sqmax=1024.0):
    """Per-row (a1,b1,g1,d1,a2,b2,g2,d2), f16-safe, and the row scale S."""
    def qabsmax(hh, rr):
        e0 = (ulo + hh) ** 2 + rr
        e1 = (uhi + hh) ** 2 + rr
        vtx = np.where((-hh >= ulo) & (-hh <= uhi), rr, e0)
        return np.maximum(np.abs(vtx), np.maximum(np.abs(e0), np.abs(e1)))

    M1 = qabsmax(h1, r1)
    M2 = qabsmax(h2, r2)
    S = np.minimum(1.0, gmax / (np.abs(c4) * M1 * M2))
    g1 = np.sign(c4) * np.sqrt(np.abs(c4) * S * M2 / M1)
    g2 = np.sqrt(np.abs(c4) * S * M1 / M2)

    def sqpeak(hh):
        return np.maximum((ulo + hh) ** 2, (uhi + hh) ** 2)

    a1 = np.sqrt(np.minimum(1.0, sqmax / sqpeak(h1)))
    a2 = np.sqrt(np.minimum(1.0, sqmax / sqpeak(h2)))
    cf = np.stack([a1, a1 * h1, g1 / a1 ** 2, g1 * r1,
                   a2, a2 * h2, g2 / a2 ** 2, g2 * r2], axis=1)
    return cf.astype(np.float32), S


def _host_prep(physical_params, eta, W1, b1, W2, b2):
    """Returns (eta_f16, cf [B,8] f32, S [B] f64)."""
    eta = np.asarray(eta, np.float32)
    cp = _stage1_coeffs(physical_params, W1, b1, W2, b2)
    h1, r1, h2, r2 = _factor_quartics(cp)
    ulo = float(eta.min()) ** 3
    uhi = float(eta.max()) ** 3
    cf, S = _pick_scales(cp[:, 4], h1, r1, h2, r2, ulo, uhi)
    return np.ascontiguousarray(eta.astype(np.float16)), cf, S


def _make_in_maps(eta16, cf):
    in_maps = []
    for g in range(NCORES):
        rows = slice(g * RPC, (g + 1) * RPC)
        in_maps.append({
            "eta": np.ascontiguousarray(eta16[rows]),
            "cf": np.ascontiguousarray(cf[rows]),
        })
    return in_maps


def kernel(physical_params, eta, W1, b1, W2, b2):
    from concourse.bass_utils import run_bass_kernel_spmd

    eta16, cf, S = _host_prep(physical_params, eta, W1, b1, W2, b2)

    if "nc" not in _cache:
        _cache["nc"] = _build_nc()
    nc = _cache["nc"]

    res = run_bass_kernel_spmd(nc, _make_in_maps(eta16, cf),
                               core_ids=list(range(NCORES)))
    _cache["last_results"] = res
    out = np.concatenate(
        [np.asarray(res.results[g]["dy"]).astype(np.float32)
         for g in range(NCORES)], axis=0)
    out /= S[:, None].astype(np.float32)
    return out
